# revision 10
# baseline (speedup 1.0000x reference)
"""Correlation cost-volume kernel for Trainium2 (8 NeuronCores, data-parallel over batch).

cost[b, d, h, w] = mean_c left[b, c, h, w] * right[b, c, h, w - d]   (0 for w < d)

Per (b, h) this is the 48-wide band of the Gram matrix G = L^T R (K = c = 128).
Pipeline per (h-group, w-block):
  Gram matmuls (PE) -> PSUM -> scaled copy to SBUF X (ACT/DVE)
  -> shear DMA (per-partition diagonal flat AP, SBUF->SBUF): S[i, hh*48+dd] = X[i, hh*175+i+dd]
  -> PE transposes of S 96-col slices -> PSUM [96, M] -> copy to O -> DMA to out[d, h, w].
"""

import sys
from contextlib import ExitStack

import numpy as np

if "/opt/trn_rl_repo" not in sys.path:
    sys.path.insert(0, "/opt/trn_rl_repo")

import concourse.bass as bass
import concourse.mybir as mybir
from concourse import bacc, tile
from concourse.ap import AP

B = 8
C = 128
H = 160
W = 320
D = 48
PAD = D - 1  # 47
HC = 16  # h rows per group
FW = PAD + 128  # 175, X stride per h row

# w-blocks: (wb, M)
WBLOCKS = [(0, 128), (128, 128), (256, 64)]


def _ncols(wb, m):
    w0 = max(0, wb - PAD)
    return min(W, wb + m) - w0, w0


def build_nc(h=H):
    nc = bacc.Bacc("TRN2", target_bir_lowering=False, debug=False)
    left_d = nc.dram_tensor("left", [C, h, W], mybir.dt.float32, kind="ExternalInput")
    right_d = nc.dram_tensor("right", [C, h, W], mybir.dt.float32, kind="ExternalInput")
    ident_d = nc.dram_tensor("ident", [128, 128], mybir.dt.float32, kind="ExternalInput")
    out_d = nc.dram_tensor("out", [D, h, W], mybir.dt.float32, kind="ExternalOutput")

    ngroups = h // HC
    hw = h * W

    with tile.TileContext(nc) as tc, ExitStack() as ctx:
        const_pool = ctx.enter_context(tc.tile_pool(name="const", bufs=1))
        lr_pool = ctx.enter_context(tc.tile_pool(name="lr", bufs=2))
        x_pool = ctx.enter_context(tc.tile_pool(name="x", bufs=2))
        s_pool = ctx.enter_context(tc.tile_pool(name="s", bufs=3))
        o_pool = ctx.enter_context(tc.tile_pool(name="o", bufs=3))
        g_pool = ctx.enter_context(tc.tile_pool(name="g", bufs=4, space="PSUM"))
        t_pool = ctx.enter_context(tc.tile_pool(name="t", bufs=3, space="PSUM"))

        ident = const_pool.tile([128, 128], mybir.dt.float32)
        nc.sync.dma_start(ident[:], ident_d[:])

        copy_parity = 0

        for g in range(ngroups):
            h0 = g * HC
            ltile = lr_pool.tile([C, HC * W], mybir.dt.float32, tag="L")
            rtile = lr_pool.tile([C, HC * W], mybir.dt.float32, tag="R")
            nc.sync.dma_start(
                ltile[:].rearrange("p (a b) -> p a b", a=HC), left_d[:, h0 : h0 + HC, :]
            )
            nc.sync.dma_start(
                rtile[:].rearrange("p (a b) -> p a b", a=HC), right_d[:, h0 : h0 + HC, :]
            )

            for wb, m in WBLOCKS:
                ncols, w0 = _ncols(wb, m)
                foff = PAD - wb + w0  # 47 for wb=0 else 0
                # X is h-interleaved: X[i, f*HC + hh] = G_hh[i, f]
                xtile = x_pool.tile([m, HC * FW], mybir.dt.float32, tag="X")

                if foff:
                    # zero-fill f < 47 region (outputs with w < d)
                    nc.gpsimd.memset(xtile[:, : foff * HC], 0.0)

                for hq in range(HC // 2):
                    gt = g_pool.tile([m, 2 * FW], mybir.dt.float32, tag="G")
                    for s in range(2):
                        hh = 2 * hq + s
                        nc.tensor.matmul(
                            gt[:, s * FW : s * FW + ncols],
                            ltile[:, hh * W + wb : hh * W + wb + m],
                            rtile[:, hh * W + w0 : hh * W + w0 + ncols],
                            start=True,
                            stop=True,
                        )
                    # PSUM -> X with 1/C scaling; dst interleaved (f stride HC)
                    dst = AP(
                        xtile[:].tensor,
                        xtile[:].offset + foff * HC + 2 * hq,
                        [[HC * FW, m], [1, 2], [HC, ncols]],
                    )
                    src = AP(gt[:].tensor, gt[:].offset, [[2 * FW, m], [FW, 2], [1, ncols]])
                    if copy_parity & 1:
                        nc.scalar.mul(dst, src, 1.0 / C)
                    else:
                        nc.vector.tensor_scalar_mul(dst, src, 1.0 / C)
                    copy_parity += 1

                # shear: S[i, dd*HC + hh] = X[i, (i+dd)*HC + hh]; per-partition
                # window is one contiguous HC*D run -> 2-dim flat AP with a
                # fused (row+byte) step. HW DGE constraints: fused-step APs
                # are only correct with offset < row width and partition
                # count not in {64, 128}. So: low chunk direct from offset 0;
                # high chunk first partition-shifted to rows [0,cnt) of a
                # scratch tile (rectangular DMA), then fused-read at offset 0.
                stile = s_pool.tile([m, HC * D], mybir.dt.float32, tag="S")
                p_lo = 96 if m == 128 else 56
                nc.sync.dma_start(
                    stile[0:p_lo, :],
                    AP(
                        xtile[:].tensor,
                        xtile[:].offset,
                        [[HC * (FW + 1), p_lo], [1, HC * D]],
                    ),
                )
                cnt = m - p_lo
                l2 = (cnt - 1) * HC + HC * D
                x2 = s_pool.tile([cnt, l2], mybir.dt.float32, tag="X2")
                nc.sync.dma_start(x2[:], xtile[p_lo:m, p_lo * HC : p_lo * HC + l2])
                nc.sync.dma_start(
                    stile[p_lo:m, :],
                    AP(x2[:].tensor, x2[:].offset, [[l2 + HC, cnt], [1, HC * D]]),
                )

                otile = o_pool.tile([6 * HC, 8 * m], mybir.dt.float32, tag="O")
                for tq in range(4):
                    tt = t_pool.tile([6 * HC, 2 * m], mybir.dt.float32, tag="T")
                    for s in range(2):
                        a = 2 * tq + s
                        # contiguous 96-col slice: (dd_l in [0,6)) x (hh in [0,16))
                        # -> out partition p = dd_l*16 + hh
                        nc.tensor.transpose(
                            tt[:, s * m : (s + 1) * m],
                            stile[:, a * 96 : (a + 1) * 96],
                            ident[:m, :m],
                        )
                    if copy_parity & 1:
                        nc.scalar.copy(otile[:, tq * 2 * m : (tq + 1) * 2 * m], tt[:])
                    else:
                        nc.vector.tensor_copy(otile[:, tq * 2 * m : (tq + 1) * 2 * m], tt[:])
                    copy_parity += 1

                # out[47-(6a+dl), h0+hh, wb+i] <- O[dl*16+hh, a*m+i]
                # one DMA per dl: dims (hh, a, i); a-stride negative (dim1)
                for dl in range(6):
                    dst = AP(
                        out_d,
                        (PAD - dl) * hw + h0 * W + wb,
                        [[W, HC], [-6 * hw, 8], [1, m]],
                    )
                    src = otile[dl * HC : (dl + 1) * HC, :].rearrange(
                        "p (a i) -> p a i", i=m
                    )
                    nc.sync.dma_start(dst, src)

    nc.compile()
    return nc


def kernel(left_feature: np.ndarray, right_feature: np.ndarray) -> np.ndarray:
    from concourse import bass_utils

    nc = build_nc()
    ident = np.eye(128, dtype=np.float32)
    in_maps = [
        {
            "left": np.ascontiguousarray(left_feature[b]),
            "right": np.ascontiguousarray(right_feature[b]),
            "ident": ident,
        }
        for b in range(B)
    ]
    res = bass_utils.run_bass_kernel_spmd(nc, in_maps, list(range(B)))
    return np.stack([res.results[b]["out"] for b in range(B)], axis=0)


# revision 15
# speedup vs baseline: 1.3082x; 1.3082x over previous
"""Correlation cost-volume kernel for Trainium2 (8 NeuronCores, data-parallel over batch).

cost[b, d, h, w] = mean_c left[b, c, h, w] * right[b, c, h, w - d]   (0 for w < d)

Per (b, h) this is the 48-wide band of the Gram matrix G = L^T R (K = c = 128).
Pipeline per (h-group of HC, w-block):
  bf16 Gram matmuls (PE) -> PSUM (4 h per 2-bank tile)
  -> scaled copy to SBUF X[i, f*HC + hh] (ACT/DVE alternate, h-interleaved)
  -> shear DMA: S[i, dd*HC + hh] = X[i, (i+dd)*HC + hh]  (flat fused-step AP;
     partition chunks avoid the HW DGE bug: counts not in {64,128}, offset 0)
  -> PE transposes of S 96-col slices -> PSUM [96, m] -> copy to O
  -> DMA to out[d, h, w] (one DMA per dd_l, negative d-stride in dim1).
"""

import sys
from contextlib import ExitStack

import numpy as np

if "/opt/trn_rl_repo" not in sys.path:
    sys.path.insert(0, "/opt/trn_rl_repo")

import concourse.bass as bass
import concourse.mybir as mybir
from concourse import bacc, tile
from concourse.ap import AP

B = 8
C = 128
H = 160
W = 320
D = 48
PAD = D - 1  # 47
HC = 32  # h rows per group
FW = PAD + 128  # 175, X f-slots per h row
MDT = mybir.dt.bfloat16  # matmul input dtype
SDT = mybir.dt.bfloat16  # X/S (post-mean) dtype

# w-blocks: (wb, M)
WBLOCKS = [(0, 128), (128, 128), (256, 64)]


def _ncols(wb, m):
    w0 = max(0, wb - PAD)
    return min(W, wb + m) - w0, w0


def build_nc(h=H):
    nc = bacc.Bacc("TRN2", target_bir_lowering=False, debug=False)
    left_d = nc.dram_tensor("left", [C, h, W], mybir.dt.float32, kind="ExternalInput")
    right_d = nc.dram_tensor("right", [C, h, W], mybir.dt.float32, kind="ExternalInput")
    ident_d = nc.dram_tensor("ident", [128, 128], mybir.dt.float32, kind="ExternalInput")
    out_d = nc.dram_tensor("out", [D, h, W], mybir.dt.float32, kind="ExternalOutput")

    ngroups = h // HC
    hw = h * W

    with tile.TileContext(nc) as tc, ExitStack() as ctx:
        const_pool = ctx.enter_context(tc.tile_pool(name="const", bufs=1))
        lr_pool = ctx.enter_context(tc.tile_pool(name="lr", bufs=2))
        x_pool = ctx.enter_context(tc.tile_pool(name="x", bufs=2))
        s_pool = ctx.enter_context(tc.tile_pool(name="s", bufs=3))
        o_pool = ctx.enter_context(tc.tile_pool(name="o", bufs=3))
        g_pool = ctx.enter_context(tc.tile_pool(name="g", bufs=2, space="PSUM"))
        t_pool = ctx.enter_context(tc.tile_pool(name="t", bufs=3, space="PSUM"))

        ident = const_pool.tile([128, 128], SDT)
        nc.gpsimd.dma_start(ident[:], ident_d[:])

        copy_parity = 0

        for g in range(ngroups):
            h0 = g * HC
            ltile = lr_pool.tile([C, HC * W], MDT, tag="L")
            rtile = lr_pool.tile([C, HC * W], MDT, tag="R")
            # SWDGE DMA with fp32 -> bf16 cast
            nc.gpsimd.dma_start(
                ltile[:].rearrange("p (a b) -> p a b", a=HC), left_d[:, h0 : h0 + HC, :]
            )
            nc.gpsimd.dma_start(
                rtile[:].rearrange("p (a b) -> p a b", a=HC), right_d[:, h0 : h0 + HC, :]
            )

            for wb, m in WBLOCKS:
                ncols, w0 = _ncols(wb, m)
                foff = PAD - wb + w0  # 47 for wb=0 else 0
                # X is h-interleaved: X[i, f*HC + hh] = G_hh[i, f]
                xtile = x_pool.tile([m, HC * FW], SDT, tag="X")

                if foff:
                    # zero-fill f < 47 region (outputs with w < d)
                    nc.gpsimd.memset(xtile[:, : foff * HC], 0.0)

                for hq in range(HC // 4):
                    # 4 matmuls per 2-bank PSUM tile at 256-slot alignment
                    gt = g_pool.tile([m, 1024], mybir.dt.float32, tag="G")
                    for s in range(4):
                        hh = 4 * hq + s
                        nc.tensor.matmul(
                            gt[:, s * 256 : s * 256 + ncols],
                            ltile[:, hh * W + wb : hh * W + wb + m],
                            rtile[:, hh * W + w0 : hh * W + w0 + ncols],
                            start=True,
                            stop=True,
                        )
                    # PSUM -> X with 1/C scaling; dst interleaved (f stride HC)
                    dst = AP(
                        xtile[:].tensor,
                        xtile[:].offset + foff * HC + 4 * hq,
                        [[HC * FW, m], [1, 4], [HC, ncols]],
                    )
                    src = AP(gt[:].tensor, gt[:].offset, [[1024, m], [256, 4], [1, ncols]])
                    if copy_parity & 1:
                        nc.scalar.mul(dst, src, 1.0 / C)
                    else:
                        nc.vector.tensor_scalar_mul(dst, src, 1.0 / C)
                    copy_parity += 1

                # shear: S[i, dd*HC + hh] = X[i, (i+dd)*HC + hh]; per-partition
                # window is one contiguous HC*D run -> 2-dim flat AP with a
                # fused (row+byte) step. HW DGE constraints: fused-step APs
                # are only correct with offset < row width and partition
                # count not in {64, 128}. Low chunk direct from offset 0;
                # high chunk partition-shifted to rows [0,cnt) of a scratch
                # tile (rectangular DMA), then fused-read at offset 0.
                stile = s_pool.tile([m, HC * D], SDT, tag="S")
                p_lo = 96 if m == 128 else 56
                nc.sync.dma_start(
                    stile[0:p_lo, :],
                    AP(
                        xtile[:].tensor,
                        xtile[:].offset,
                        [[HC * (FW + 1), p_lo], [1, HC * D]],
                    ),
                )
                cnt = m - p_lo
                l2 = (cnt - 1) * HC + HC * D
                x2 = s_pool.tile([cnt, l2], SDT, tag="X2")
                nc.sync.dma_start(x2[:], xtile[p_lo:m, p_lo * HC : p_lo * HC + l2])
                nc.sync.dma_start(
                    stile[p_lo:m, :],
                    AP(x2[:].tensor, x2[:].offset, [[l2 + HC, cnt], [1, HC * D]]),
                )

                # transposes: 96-col slices (3 dd x 32 hh) -> p = dd_l*32 + hh
                otile = o_pool.tile([3 * HC, 16 * m], mybir.dt.float32, tag="O")
                for tq in range(4):
                    tt = t_pool.tile([3 * HC, 4 * m], SDT, tag="T")
                    for s in range(4):
                        a = 4 * tq + s
                        nc.tensor.transpose(
                            tt[:, s * m : (s + 1) * m],
                            stile[:, a * 96 : (a + 1) * 96],
                            ident[:m, :m],
                        )
                    if copy_parity & 1:
                        nc.scalar.copy(otile[:, tq * 4 * m : (tq + 1) * 4 * m], tt[:])
                    else:
                        nc.vector.tensor_copy(otile[:, tq * 4 * m : (tq + 1) * 4 * m], tt[:])
                    copy_parity += 1

                # out[47-(3a+dl), h0+hh, wb+i] <- O[dl*32+hh, a*m+i]
                # one DMA per dl: dims (hh, a, i); a-stride negative (dim1)
                for dl in range(3):
                    dst = AP(
                        out_d,
                        (PAD - dl) * hw + h0 * W + wb,
                        [[W, HC], [-3 * hw, 16], [1, m]],
                    )
                    src = otile[dl * HC : (dl + 1) * HC, :].rearrange(
                        "p (a i) -> p a i", i=m
                    )
                    nc.sync.dma_start(dst, src)

    nc.compile()
    return nc


def kernel(left_feature: np.ndarray, right_feature: np.ndarray) -> np.ndarray:
    from concourse import bass_utils

    nc = build_nc()
    ident = np.eye(128, dtype=np.float32)
    in_maps = [
        {
            "left": np.ascontiguousarray(left_feature[b]),
            "right": np.ascontiguousarray(right_feature[b]),
            "ident": ident,
        }
        for b in range(B)
    ]
    res = bass_utils.run_bass_kernel_spmd(nc, in_maps, list(range(B)))
    return np.stack([res.results[b]["out"] for b in range(B)], axis=0)


# revision 19
# speedup vs baseline: 1.6452x; 1.2577x over previous
"""Correlation cost-volume kernel for Trainium2 (8 NeuronCores, data-parallel over batch).

cost[b, d, h, w] = mean_c left[b, c, h, w] * right[b, c, h, w - d]   (0 for w < d)

Per (b, h) this is the 48-wide band of the Gram matrix G = L^T R (K = c = 128).
Pipeline per (h-group of HC, w-block):
  bf16 Gram matmuls (PE) -> PSUM (4 h per 2-bank tile)
  -> scaled copy to SBUF X[i, f*HC + hh] (ACT/DVE alternate, h-interleaved)
  -> shear DMA: S[i, dd*HC + hh] = X[i, (i+dd)*HC + hh]  (flat fused-step AP;
     partition chunks avoid the HW DGE bug: counts not in {64,128}, offset 0)
  -> PE transposes of S 96-col slices -> PSUM [96, m] -> copy to O
  -> DMA to out[d, h, w] (one DMA per dd_l, negative d-stride in dim1).
"""

import sys
from contextlib import ExitStack

import numpy as np

if "/opt/trn_rl_repo" not in sys.path:
    sys.path.insert(0, "/opt/trn_rl_repo")

import concourse.bass as bass
import concourse.mybir as mybir
from concourse import bacc, tile
from concourse.ap import AP

B = 8
C = 128
H = 160
W = 320
D = 48
PAD = D - 1  # 47
HC = 32  # h rows per group
FW = PAD + 128  # 175, X f-slots per h row
MDT = mybir.dt.bfloat16  # matmul input dtype
SDT = mybir.dt.bfloat16  # X/S (post-mean) dtype

# w-blocks: (wb, M)
WBLOCKS = [(0, 128), (128, 128), (256, 64)]


def _ncols(wb, m):
    w0 = max(0, wb - PAD)
    return min(W, wb + m) - w0, w0


def build_nc(h=H):
    nc = bacc.Bacc("TRN2", target_bir_lowering=False, debug=False)
    left_d = nc.dram_tensor("left", [C, h, W], mybir.dt.float32, kind="ExternalInput")
    right_d = nc.dram_tensor("right", [C, h, W], mybir.dt.float32, kind="ExternalInput")
    ident_d = nc.dram_tensor("ident", [128, 128], mybir.dt.float32, kind="ExternalInput")
    out_d = nc.dram_tensor("out", [D, h, W], mybir.dt.float32, kind="ExternalOutput")

    ngroups = h // HC
    hw = h * W

    with tile.TileContext(nc) as tc, ExitStack() as ctx:
        const_pool = ctx.enter_context(tc.tile_pool(name="const", bufs=1))
        lr_pool = ctx.enter_context(tc.tile_pool(name="lr", bufs=2))
        x_pool = ctx.enter_context(tc.tile_pool(name="x", bufs=2))
        s_pool = ctx.enter_context(tc.tile_pool(name="s", bufs=2))
        o_pool = ctx.enter_context(tc.tile_pool(name="o", bufs=2))
        g_pool = ctx.enter_context(tc.tile_pool(name="g", bufs=2, space="PSUM"))
        t_pool = ctx.enter_context(tc.tile_pool(name="t", bufs=3, space="PSUM"))

        ident = const_pool.tile([128, 128], SDT)
        nc.gpsimd.dma_start(ident[:], ident_d[:])

        copy_parity = 0

        for g in range(ngroups):
            h0 = g * HC
            ltile = lr_pool.tile([C, HC * W], MDT, tag="L")
            rtile = lr_pool.tile([C, HC * W], MDT, tag="R")
            # SWDGE DMA with fp32 -> bf16 cast
            nc.gpsimd.dma_start(
                ltile[:].rearrange("p (a b) -> p a b", a=HC), left_d[:, h0 : h0 + HC, :]
            )
            nc.gpsimd.dma_start(
                rtile[:].rearrange("p (a b) -> p a b", a=HC), right_d[:, h0 : h0 + HC, :]
            )

            # O spans all 3 w-blocks so out-DMA runs are full 1280B w rows
            otile = o_pool.tile([3 * HC, 16 * W], mybir.dt.float32, tag="O")

            for wb, m in WBLOCKS:
                ncols, w0 = _ncols(wb, m)
                foff = PAD - wb + w0  # 47 for wb=0 else 0
                # X is h-interleaved: X[i, f*HC + hh] = G_hh[i, f]
                xtile = x_pool.tile([m, HC * FW], mybir.dt.float32, tag="X")

                if foff:
                    # zero-fill f < 47 region (outputs with w < d)
                    nc.gpsimd.memset(xtile[:, : foff * HC], 0.0)

                for hq in range(HC // 4):
                    # 4 matmuls per 2-bank PSUM tile at 256-slot alignment
                    gt = g_pool.tile([m, 1024], mybir.dt.float32, tag="G")
                    for s in range(4):
                        hh = 4 * hq + s
                        nc.tensor.matmul(
                            gt[:, s * 256 : s * 256 + ncols],
                            ltile[:, hh * W + wb : hh * W + wb + m],
                            rtile[:, hh * W + w0 : hh * W + w0 + ncols],
                            start=True,
                            stop=True,
                        )
                    # PSUM -> X with 1/C scaling; dst interleaved (f stride HC)
                    dst = AP(
                        xtile[:].tensor,
                        xtile[:].offset + foff * HC + 4 * hq,
                        [[HC * FW, m], [1, 4], [HC, ncols]],
                    )
                    src = AP(gt[:].tensor, gt[:].offset, [[1024, m], [256, 4], [1, ncols]])
                    if copy_parity & 1:
                        nc.scalar.mul(dst, src, 1.0 / C)
                    else:
                        nc.vector.tensor_scalar_mul(dst, src, 1.0 / C)
                    copy_parity += 1

                # shear: S[i, dd*HC + hh] = X[i, (i+dd)*HC + hh]; per-partition
                # window is one contiguous HC*D run -> 2-dim flat AP with a
                # fused (row+byte) step. HW DGE constraints: fused-step APs
                # are only correct with offset < row width and partition
                # count not in {64, 128}. Low chunk direct from offset 0;
                # high chunk partition-shifted to rows [0,cnt) of a scratch
                # tile (rectangular DMA), then fused-read at offset 0.
                stile = s_pool.tile([m, HC * D], SDT, tag="S")
                p_lo = 96 if m == 128 else 56
                # SWDGE: fp32 -> bf16 cast during the shear reads
                nc.gpsimd.dma_start(
                    stile[0:p_lo, :],
                    AP(
                        xtile[:].tensor,
                        xtile[:].offset,
                        [[HC * (FW + 1), p_lo], [1, HC * D]],
                    ),
                )
                cnt = m - p_lo
                l2 = (cnt - 1) * HC + HC * D
                x2 = s_pool.tile([cnt, l2], SDT, tag="X2")
                nc.gpsimd.dma_start(x2[:], xtile[p_lo:m, p_lo * HC : p_lo * HC + l2])
                nc.sync.dma_start(
                    stile[p_lo:m, :],
                    AP(x2[:].tensor, x2[:].offset, [[l2 + HC, cnt], [1, HC * D]]),
                )

                # transposes: 96-col slices (3 dd x 32 hh) -> p = dd_l*32 + hh
                for tq in range(4):
                    tt = t_pool.tile([3 * HC, 4 * m], SDT, tag="T")
                    for s in range(4):
                        a = 4 * tq + s
                        nc.tensor.transpose(
                            tt[:, s * m : (s + 1) * m],
                            stile[:, a * 96 : (a + 1) * 96],
                            ident[:m, :m],
                        )
                    # O[p, a*W + wb + i] <- T[p, (a-4*tq)*m + i]
                    dst = AP(
                        otile[:].tensor,
                        otile[:].offset + 4 * tq * W + wb,
                        [[16 * W, 3 * HC], [W, 4], [1, m]],
                    )
                    if copy_parity & 1:
                        nc.scalar.copy(dst, tt[:].rearrange("p (a i) -> p a i", i=m))
                    else:
                        nc.vector.tensor_copy(dst, tt[:].rearrange("p (a i) -> p a i", i=m))
                    copy_parity += 1

            # out[47-(3a+dl), h0+hh, w] <- O[dl*32+hh, a*W+w]
            # one DMA per dl: dims (hh, a, w); a-stride negative (dim1)
            for dl in range(3):
                dst = AP(
                    out_d,
                    (PAD - dl) * hw + h0 * W,
                    [[W, HC], [-3 * hw, 16], [1, W]],
                )
                src = otile[dl * HC : (dl + 1) * HC, :].rearrange(
                    "p (a w) -> p a w", w=W
                )
                nc.sync.dma_start(dst, src)

    nc.compile()
    return nc


def kernel(left_feature: np.ndarray, right_feature: np.ndarray) -> np.ndarray:
    from concourse import bass_utils

    nc = build_nc()
    ident = np.eye(128, dtype=np.float32)
    in_maps = [
        {
            "left": np.ascontiguousarray(left_feature[b]),
            "right": np.ascontiguousarray(right_feature[b]),
            "ident": ident,
        }
        for b in range(B)
    ]
    res = bass_utils.run_bass_kernel_spmd(nc, in_maps, list(range(B)))
    return np.stack([res.results[b]["out"] for b in range(B)], axis=0)
